# revision 12
# baseline (speedup 1.0000x reference)
"""Cached multi-head attention (decode, T=8) Bass/Tile kernel for 8 TRN2 cores.

Problem (hardcoded):
  query_input [8, 16, 1024] f32, prev_key/prev_value [16, 16, 4096, 64] f32,
  key_padding_mask [16, 4096] i32, new_state_order [16] i32,
  Wq/Wo [1024, 1024] f32, bq/bo [1024] f32.
  out = MHA(q=(x@Wq.T+bq)*hd^-0.5, k=pk[order], v=pv[order], additive -FMAX mask) @ Wo.T + bo

Sharding: data parallel over batch, 2 batches per core. The KV beam reorder
(gather over batch) is folded into the host-side shard slicing. Wq is
pre-transposed and pre-scaled on the host (exact: scale is a power of two);
Wo pre-transposed; biases pre-broadcast.

Per-core device kernel (all f32):
  - q-proj: out[bt=16, j] = sum_i xT[i, bt] * WqT[i, j], weights stationary.
  - qT per head via PE transpose (matmul with identity), assembled into a
    block-diagonal lhsT "qbd" [128=(2 heads x 64 d), 32=(2 b x 2 heads x 8 t)]
    so one K=128 QK matmul computes TWO heads' scores at once.
  - K tiles [128 s, 128=(2 heads x 64 d)] are PE-transposed to kT pair tiles
    [128=(2h x d), 128 s]; QK streams them N=512 per chunk.
  - scores [16=(2h x 8t), 512] land at 32-aligned PSUM partition offsets
    (hardware constraint), 4 head-pairs per bank -> 2 banks per s-chunk.
  - mask: one rank-1 accumulating matmul ones[1,128] x (-FMAX*mask)[1,512]
    adds the additive mask to every row; exact vs reference (|-FMAX| >> |s|).
  - softmax without max-subtraction (scores are O(10), exp cannot overflow);
    exp on ACT engine with accum_out producing per-chunk row sums.
  - two-pass: after all chunks, rowsum -> reciprocal -> scale attn in SBUF.
  - AV: attn chunks PE-transposed ([s, ht] layout), then per (head, 128-s
    piece) matmuls accumulate attn_out [8 t, 16 h x 64 d] in two PSUM banks.
  - out-proj: attn_out transposed to [(h d), t], then weights-stationary
    matmuls vs WoT chunks; bias added via pre-broadcast tile; DMA out.
"""

import os
from contextlib import ExitStack

import numpy as np

import concourse.bacc as bacc
import concourse.bass as bass
import concourse.tile as tile
from concourse import mybir
from concourse.bass_utils import run_bass_kernel_spmd

F32 = mybir.dt.float32
FMAX = float(np.finfo(np.float32).max)
SCALE = 64.0 ** -0.5  # HEAD_DIM ** -0.5 = 0.125

T = 8            # tgt len
B = 16           # full batch
S = 4096         # cached src len
H = 1024         # hidden
NH = 16          # heads
HD = 64          # head dim
NCORES = 8
BPC = B // NCORES        # 2 batches per core
SCH = 512                # s chunk
NCHUNK = S // SCH        # 8
NPIECE = SCH // 128      # 4
NHP = NH // 2            # 8 head pairs

Exp = mybir.ActivationFunctionType.Exp
AxX = mybir.AxisListType.X


def _emit(ctx: ExitStack, tc: tile.TileContext, ins, outt):
    nc = tc.nc
    xt, k_in, v_in, mb, wqt, wot, bqbc, bobc, ident = (
        ins["xt"], ins["kin"], ins["vin"], ins["mb"], ins["wqt"], ins["wot"],
        ins["bqbc"], ins["bobc"], ins["ident"],
    )

    const = ctx.enter_context(tc.tile_pool(name="const", bufs=1))
    wpool = ctx.enter_context(tc.tile_pool(name="wpool", bufs=1))
    kin = ctx.enter_context(tc.tile_pool(name="kin", bufs=6))
    ktsb = ctx.enter_context(tc.tile_pool(name="ktsb", bufs=3))
    vin = ctx.enter_context(tc.tile_pool(name="vin", bufs=12))
    apool = ctx.enter_context(tc.tile_pool(name="apool", bufs=18))
    atp = ctx.enter_context(tc.tile_pool(name="atp", bufs=2))
    wk = ctx.enter_context(tc.tile_pool(name="wk", bufs=2))
    psum = ctx.enter_context(tc.tile_pool(name="psum", bufs=2, space="PSUM"))

    # ---- constants / weights ----
    ident_sb = const.tile([128, 128], F32)
    nc.sync.dma_start(ident_sb[:], ident[:])
    ones_sb = const.tile([1, 128], F32)
    nc.gpsimd.memset(ones_sb[:], 1.0)
    mb_sb = const.tile([1, BPC * S], F32)
    nc.sync.dma_start(mb_sb[:], mb[:])
    bq_sb = const.tile([BPC * T, H], F32)
    nc.sync.dma_start(bq_sb[:], bqbc[:])
    bo_sb = const.tile([T, H], F32)
    nc.sync.dma_start(bo_sb[:], bobc[:])
    xt_sb = const.tile([128, 8 * BPC * T], F32)
    nc.sync.dma_start(
        xt_sb[:].rearrange("p (ic bt) -> p ic bt", ic=8),
        xt.rearrange("(ic p) bt -> p ic bt", p=128),
    )
    wqt_sb = []
    wot_sb = []
    for i in range(8):
        wq_t = wpool.tile([128, H], F32, tag=f"wqt{i}", name=f"wqt{i}")
        nc.sync.dma_start(wq_t[:], wqt[128 * i : 128 * (i + 1), :])
        wqt_sb.append(wq_t)
        wo_t = wpool.tile([128, H], F32, tag=f"wot{i}", name=f"wot{i}")
        nc.sync.dma_start(wo_t[:], wot[128 * i : 128 * (i + 1), :])
        wot_sb.append(wo_t)

    # ---- q projection: q[bt, j] = sum_i xT[i, bt] wqt[i, j]  (+bias) ----
    q_sb = const.tile([BPC * T, H], F32)
    for jb in range(2):
        qp = psum.tile([BPC * T, 512], F32, tag="sc", name=f"qproj{jb}")
        for i in range(8):
            nc.tensor.matmul(
                qp[:],
                lhsT=xt_sb[:, 16 * i : 16 * (i + 1)],
                rhs=wqt_sb[i][:, 512 * jb : 512 * (jb + 1)],
                start=(i == 0),
                stop=(i == 7),
            )
        nc.vector.tensor_add(
            q_sb[:, 512 * jb : 512 * (jb + 1)], qp[:], bq_sb[:, 512 * jb : 512 * (jb + 1)]
        )

    # ---- qT per head -> block-diagonal qbd [128, NHP*2*32] ----
    # qbd cols: pair hp, batch b block of 32 = [side(2) x t(8), 16 zero-pad];
    # col = 64*hp + 32*b + 8*side + t ; rows = 64*side + d. Off-block zero.
    # The 16 zero-pad columns make each QK matmul M=32, so the four slot
    # matmuls (start=True each) cover all 128 partitions of the scores bank
    # and the full-bank mask matmul can accumulate (per-partition group rule).
    qbd = const.tile([128, NHP * BPC * 32], F32)
    nc.vector.memset(qbd[:], 0.0)
    qt_ps = psum.tile([128, 128], F32, tag="ktp", name="qt_ps")
    for h in range(NH):
        side = h % 2
        hp = h // 2
        nc.tensor.matmul(
            qt_ps[64 * side : 64 * side + 64, 16 * hp : 16 * hp + 16],
            lhsT=q_sb[:, HD * h : HD * (h + 1)],
            rhs=ident_sb[0 : BPC * T, 0 : BPC * T],
            start=True,
            stop=True,
        )
    for h in range(NH):
        side = h % 2
        hp = h // 2
        src = (
            qt_ps[64 * side : 64 * side + 64, :]
            .rearrange("p (hp b t) -> p hp b t", hp=NHP, b=BPC)[:, hp, :, :]
        )
        dst = (
            qbd[64 * side : 64 * side + 64, :]
            .rearrange("p (hp b s t) -> p hp b s t", hp=NHP, b=BPC, s=4)[:, hp, :, side, :]
        )
        nc.scalar.copy(dst, src)

    # ---- main loops over the two batches ----
    for b in range(BPC):
        kdram = k_in[b]
        vdram = v_in[b]

        attn_tiles = [[None] * 2 for _ in range(NCHUNK)]
        sums = [wk.tile([128, NCHUNK], F32, tag=f"sums{bk}", name=f"sums{b}_{bk}")
                for bk in range(2)]
        sc = [None, None]

        # -- pass 1: scores + exp per chunk --
        for c in range(NCHUNK):
            for hp in range(NHP):
                bk, slot = hp // 4, hp % 4
                ktile = kin.tile([128, SCH], F32, tag="kin", name=f"ktile{b}_{c}_{hp}")
                ktv = ktile[:].rearrange("p (np q) -> p np q", np=NPIECE)
                for side in range(2):
                    h = 2 * hp + side
                    nc.sync.dma_start(
                        ktv[:, :, 64 * side : 64 * side + 64],
                        kdram[h].rearrange("(cc p) d -> p cc d", p=128)[
                            :, NPIECE * c : NPIECE * (c + 1), :
                        ],
                    )
                ktp = psum.tile([128, SCH], F32, tag="ktp", name=f"ktp{b}_{c}_{hp}")
                for p in range(NPIECE):
                    nc.tensor.matmul(
                        ktp[:, 128 * p : 128 * (p + 1)],
                        lhsT=ktile[:, 128 * p : 128 * (p + 1)],
                        rhs=ident_sb[:],
                        start=True,
                        stop=True,
                    )
                kts = ktsb.tile([128, SCH], F32, tag="kts", name=f"kts{b}_{c}_{hp}")
                if (hp + c) % 2 == 0:
                    nc.scalar.copy(kts[:], ktp[:])
                else:
                    nc.vector.tensor_copy(kts[:], ktp[:])

                if slot == 0:
                    sc[bk] = psum.tile([128, SCH], F32, tag="sc", name=f"sc{b}_{c}_{bk}")
                nc.tensor.matmul(
                    sc[bk][32 * slot : 32 * slot + 32, :],
                    lhsT=qbd[:, 64 * hp + 32 * b : 64 * hp + 32 * b + 32],
                    rhs=kts[:],
                    start=True,
                    stop=False,
                    tile_position=(0, 32 * slot),
                    # sim-only: the global group-check misaddresses
                    # partition-offset psum outs; data semantics are exact.
                    skip_group_check=True,
                )
                if slot == 3:
                    nc.tensor.matmul(
                        sc[bk][:],
                        lhsT=ones_sb[:],
                        rhs=mb_sb[0:1, S * b + SCH * c : S * b + SCH * (c + 1)],
                        start=False,
                        stop=True,
                        skip_group_check=True,
                    )
                    at = apool.tile([128, SCH], F32, tag="attn",
                                    name=f"attn{b}_{c}_{bk}")
                    nc.scalar.activation(
                        at[:], sc[bk][:], Exp, accum_out=sums[bk][:, c : c + 1]
                    )
                    attn_tiles[c][bk] = at

        # -- normalize --
        rcp = [wk.tile([128, 1], F32, tag=f"rcp{bk}", name=f"rcp{b}_{bk}")
               for bk in range(2)]
        for bk in range(2):
            tot = wk.tile([128, 1], F32, tag=f"tot{bk}", name=f"tot{b}_{bk}")
            nc.vector.reduce_sum(tot[:], sums[bk][:], axis=AxX)
            nc.vector.reciprocal(rcp[bk][:], tot[:])
        for c in range(NCHUNK):
            for bk in range(2):
                nc.vector.tensor_scalar_mul(
                    attn_tiles[c][bk][:], attn_tiles[c][bk][:], rcp[bk][:]
                )

        # -- pass 2: attn^T + AV --
        av = [
            psum.tile([T, 512], F32, tag="av", name=f"av{b}_{bk}")
            for bk in range(2)
        ]
        for c in range(NCHUNK):
            for h in range(NH):
                vt = vin.tile([128, NPIECE * HD], F32, tag="vin",
                              name=f"vt{b}_{c}_{h}")
                nc.sync.dma_start(
                    vt[:].rearrange("p (np d) -> p np d", np=NPIECE),
                    vdram[h].rearrange("(cc p) d -> p cc d", p=128)[
                        :, NPIECE * c : NPIECE * (c + 1), :
                    ],
                )
                if h == 0:
                    atps = [None, None]
                    att_sb = [None, None]
                    for bk in range(2):
                        atps[bk] = psum.tile([128, SCH], F32, tag="attnT",
                                             name=f"atp{b}_{c}_{bk}")
                        for p in range(NPIECE):
                            nc.tensor.matmul(
                                atps[bk][:, 128 * p : 128 * (p + 1)],
                                lhsT=attn_tiles[c][bk][:, 128 * p : 128 * (p + 1)],
                                rhs=ident_sb[:],
                                start=True,
                                stop=True,
                            )
                        att_sb[bk] = atp.tile([128, SCH], F32, tag="attnT_sb",
                                              name=f"atsb{b}_{c}_{bk}")
                        nc.vector.tensor_copy(att_sb[bk][:], atps[bk][:])
                hp, side = h // 2, h % 2
                bk, slot = hp // 4, hp % 4
                for p in range(NPIECE):
                    nc.tensor.matmul(
                        av[h // 8][:, 64 * (h % 8) : 64 * (h % 8) + 64],
                        lhsT=att_sb[bk][
                            :, 128 * p + 32 * slot + 8 * side : 128 * p + 32 * slot + 8 * side + 8
                        ],
                        rhs=vt[:, HD * p : HD * (p + 1)],
                        start=(c == 0 and p == 0 and h % 8 == 0),
                        stop=(c == NCHUNK - 1 and p == NPIECE - 1 and h % 8 == 7),
                    )

        # -- attn_out -> [(h d), t] -> out-proj --
        ao_sb = wk.tile([T, H], F32, tag="ao", name=f"ao{b}")
        for bk in range(2):
            nc.scalar.copy(ao_sb[:, 512 * bk : 512 * (bk + 1)], av[bk][:])
        aot_ps = psum.tile([128, HD], F32, tag="attnT", name=f"aotp{b}")
        for jc in range(8):
            nc.tensor.matmul(
                aot_ps[:, 8 * jc : 8 * (jc + 1)],
                lhsT=ao_sb[:, 128 * jc : 128 * (jc + 1)],
                rhs=ident_sb[0:T, 0:T],
                start=True,
                stop=True,
            )
        aot_sb = wk.tile([128, HD], F32, tag="aot", name=f"aot{b}")
        nc.scalar.copy(aot_sb[:], aot_ps[:])
        out_sb = wk.tile([T, H], F32, tag="out", name=f"out{b}")
        for ob in range(2):
            op = psum.tile([T, 512], F32, tag="av", name=f"outp{b}_{ob}")
            for jc in range(8):
                nc.tensor.matmul(
                    op[:],
                    lhsT=aot_sb[:, 8 * jc : 8 * (jc + 1)],
                    rhs=wot_sb[jc][:, 512 * ob : 512 * (ob + 1)],
                    start=(jc == 0),
                    stop=(jc == 7),
                )
            nc.vector.tensor_add(
                out_sb[:, 512 * ob : 512 * (ob + 1)], op[:],
                bo_sb[:, 512 * ob : 512 * (ob + 1)],
            )
        nc.sync.dma_start(outt[b], out_sb[:])


def _build():
    nc = bacc.Bacc("TRN2", target_bir_lowering=False, debug=False)
    ins = {}
    ins["xt"] = nc.dram_tensor("xt", [H, BPC * T], F32, kind="ExternalInput").ap()
    kin = []
    vin = []
    for b in range(BPC):
        kin.append(nc.dram_tensor(f"k{b}", [NH, S, HD], F32, kind="ExternalInput").ap())
        vin.append(nc.dram_tensor(f"v{b}", [NH, S, HD], F32, kind="ExternalInput").ap())
    ins["kin"] = kin
    ins["vin"] = vin
    ins["mb"] = nc.dram_tensor("mb", [1, BPC * S], F32, kind="ExternalInput").ap()
    ins["wqt"] = nc.dram_tensor("wqt", [H, H], F32, kind="ExternalInput").ap()
    ins["wot"] = nc.dram_tensor("wot", [H, H], F32, kind="ExternalInput").ap()
    ins["bqbc"] = nc.dram_tensor("bqbc", [BPC * T, H], F32, kind="ExternalInput").ap()
    ins["bobc"] = nc.dram_tensor("bobc", [T, H], F32, kind="ExternalInput").ap()
    ins["ident"] = nc.dram_tensor("ident", [128, 128], F32, kind="ExternalInput").ap()
    outt = nc.dram_tensor("outt", [BPC, T, H], F32, kind="ExternalOutput").ap()
    with tile.TileContext(nc) as tc:
        with ExitStack() as ctx:
            _emit(ctx, tc, ins, outt)
    nc.compile()
    return nc


_NC_CACHE = {}
LAST_RESULT = None


def _get_nc():
    if "nc" not in _NC_CACHE:
        _NC_CACHE["nc"] = _build()
    return _NC_CACHE["nc"]


def kernel(query_input, prev_key, prev_value, key_padding_mask, new_state_order,
           Wq, bq, Wo, bo):
    global LAST_RESULT
    nc = _get_nc()

    q = np.ascontiguousarray(np.asarray(query_input, dtype=np.float32))
    pk = np.asarray(prev_key)
    pv = np.asarray(prev_value)
    order = np.asarray(new_state_order).astype(np.int64)
    mask = np.asarray(key_padding_mask)

    wqt = np.ascontiguousarray(np.asarray(Wq, dtype=np.float32).T * np.float32(SCALE))
    wot = np.ascontiguousarray(np.asarray(Wo, dtype=np.float32).T)
    bqbc = np.ascontiguousarray(
        np.broadcast_to(np.asarray(bq, dtype=np.float32) * np.float32(SCALE),
                        (BPC * T, H))
    )
    bobc = np.ascontiguousarray(np.broadcast_to(np.asarray(bo, dtype=np.float32), (T, H)))
    ident = np.eye(128, dtype=np.float32)
    mbfull = (-np.float32(FMAX)) * mask.astype(np.float32)  # [16, 4096]

    in_maps = []
    for core in range(NCORES):
        bids = [BPC * core + i for i in range(BPC)]
        im = {
            "xt": np.ascontiguousarray(
                np.transpose(q[:, bids, :], (2, 1, 0)).reshape(H, BPC * T)
            ),
            "mb": np.ascontiguousarray(mbfull[bids].reshape(1, BPC * S)),
            "wqt": wqt, "wot": wot, "bqbc": bqbc, "bobc": bobc, "ident": ident,
        }
        for i, bi in enumerate(bids):
            im[f"k{i}"] = np.ascontiguousarray(pk[order[bi]].astype(np.float32, copy=False))
            im[f"v{i}"] = np.ascontiguousarray(pv[order[bi]].astype(np.float32, copy=False))
        in_maps.append(im)

    res = run_bass_kernel_spmd(nc, in_maps, core_ids=list(range(NCORES)))
    LAST_RESULT = res

    out = np.empty((T, B, H), dtype=np.float32)
    for core in range(NCORES):
        o = res.results[core]["outt"]  # [BPC, T, H]
        for i in range(BPC):
            out[:, BPC * core + i, :] = o[i]
    return out


# revision 21
# speedup vs baseline: 1.1356x; 1.1356x over previous
"""Cached multi-head attention (decode, T=8) Bass/Tile kernel for 8 TRN2 cores.

Problem (hardcoded):
  query_input [8, 16, 1024] f32, prev_key/prev_value [16, 16, 4096, 64] f32,
  key_padding_mask [16, 4096] i32, new_state_order [16] i32,
  Wq/Wo [1024, 1024] f32, bq/bo [1024] f32.
  out = MHA(q=(x@Wq.T+bq)*hd^-0.5, k=pk[order], v=pv[order], additive -FMAX mask) @ Wo.T + bo

Sharding: data parallel over batch, 2 batches per core. The KV beam reorder
(gather over batch) is folded into the host-side shard slicing. Wq is
pre-transposed and pre-scaled on the host (exact: scale is a power of two);
Wo pre-transposed; biases pre-broadcast.

Per-core device kernel (all f32):
  - q-proj: out[bt=16, j] = sum_i xT[i, bt] * WqT[i, j], weights stationary.
  - qT per head via PE transpose (matmul with identity), assembled into a
    block-diagonal lhsT "qbd" [128=(2 heads x 64 d), 32=(2 b x 2 heads x 8 t)]
    so one K=128 QK matmul computes TWO heads' scores at once.
  - K tiles [128 s, 128=(2 heads x 64 d)] are PE-transposed to kT pair tiles
    [128=(2h x d), 128 s]; QK streams them N=512 per chunk.
  - scores [16=(2h x 8t), 512] land at 32-aligned PSUM partition offsets
    (hardware constraint), 4 head-pairs per bank -> 2 banks per s-chunk.
  - mask: one rank-1 accumulating matmul ones[1,128] x (-FMAX*mask)[1,512]
    adds the additive mask to every row; exact vs reference (|-FMAX| >> |s|).
  - softmax without max-subtraction (scores are O(10), exp cannot overflow);
    exp on ACT engine with accum_out producing per-chunk row sums.
  - two-pass: after all chunks, rowsum -> reciprocal -> scale attn in SBUF.
  - AV: attn chunks PE-transposed ([s, ht] layout), then per (head, 128-s
    piece) matmuls accumulate attn_out [8 t, 16 h x 64 d] in two PSUM banks.
  - out-proj: attn_out transposed to [(h d), t], then weights-stationary
    matmuls vs WoT chunks; bias added via pre-broadcast tile; DMA out.
"""

import os
from contextlib import ExitStack

import numpy as np

import concourse.bacc as bacc
import concourse.bass as bass
import concourse.tile as tile
from concourse import mybir
from concourse.bass_utils import run_bass_kernel_spmd

F32 = mybir.dt.float32
FMAX = float(np.finfo(np.float32).max)
SCALE = 64.0 ** -0.5  # HEAD_DIM ** -0.5 = 0.125

T = 8            # tgt len
B = 16           # full batch
S = 4096         # cached src len
H = 1024         # hidden
NH = 16          # heads
HD = 64          # head dim
NCORES = 8
BPC = B // NCORES        # 2 batches per core
SCH = 512                # s chunk
NCHUNK = S // SCH        # 8
NPIECE = SCH // 128      # 4
NHP = NH // 2            # 8 head pairs

Exp = mybir.ActivationFunctionType.Exp
AxX = mybir.AxisListType.X


def _emit(ctx: ExitStack, tc: tile.TileContext, ins, outt):
    nc = tc.nc
    xt, k_in, v_in, mb, wqt, wot, bqbc, bobc, ident = (
        ins["xt"], ins["kin"], ins["vin"], ins["mb"], ins["wqt"], ins["wot"],
        ins["bqbc"], ins["bobc"], ins["ident"],
    )

    const = ctx.enter_context(tc.tile_pool(name="const", bufs=1))
    wpool = ctx.enter_context(tc.tile_pool(name="wpool", bufs=1))
    ktsb = ctx.enter_context(tc.tile_pool(name="ktsb", bufs=4))
    vin = ctx.enter_context(tc.tile_pool(name="vin", bufs=8))
    apool = ctx.enter_context(tc.tile_pool(name="apool", bufs=18))
    atp = ctx.enter_context(tc.tile_pool(name="atp", bufs=2))
    wk = ctx.enter_context(tc.tile_pool(name="wk", bufs=2))
    psum = ctx.enter_context(tc.tile_pool(name="psum", bufs=2, space="PSUM"))

    # ---- constants / weights ----
    ident_sb = const.tile([128, 128], F32)
    nc.sync.dma_start(ident_sb[:], ident[:])
    ones_sb = const.tile([1, 128], F32)
    nc.gpsimd.memset(ones_sb[:], 1.0)
    mb_sb = const.tile([1, BPC * S], F32)
    nc.sync.dma_start(mb_sb[:], mb[:])
    bq_sb = const.tile([BPC * T, H], F32)
    nc.sync.dma_start(bq_sb[:], bqbc[:])
    bo_sb = const.tile([T, H], F32)
    nc.sync.dma_start(bo_sb[:], bobc[:])
    xt_sb = const.tile([128, 8 * BPC * T], F32)
    nc.sync.dma_start(
        xt_sb[:].rearrange("p (ic bt) -> p ic bt", ic=8),
        xt.rearrange("(ic p) bt -> p ic bt", p=128),
    )
    wqt_sb = []
    wot_sb = []
    for i in range(8):
        wq_t = wpool.tile([128, H], F32, tag=f"wqt{i}", name=f"wqt{i}")
        nc.sync.dma_start(wq_t[:], wqt[128 * i : 128 * (i + 1), :])
        wqt_sb.append(wq_t)
        wo_t = wpool.tile([128, H], F32, tag=f"wot{i}", name=f"wot{i}")
        nc.sync.dma_start(wo_t[:], wot[128 * i : 128 * (i + 1), :])
        wot_sb.append(wo_t)

    # ---- q projection: q[bt, j] = sum_i xT[i, bt] wqt[i, j]  (+bias) ----
    q_sb = const.tile([BPC * T, H], F32)
    for jb in range(2):
        qp = psum.tile([BPC * T, 512], F32, tag="sc", bufs=4, name=f"qproj{jb}")
        for i in range(8):
            nc.tensor.matmul(
                qp[:],
                lhsT=xt_sb[:, 16 * i : 16 * (i + 1)],
                rhs=wqt_sb[i][:, 512 * jb : 512 * (jb + 1)],
                start=(i == 0),
                stop=(i == 7),
            )
        nc.vector.tensor_add(
            q_sb[:, 512 * jb : 512 * (jb + 1)], qp[:], bq_sb[:, 512 * jb : 512 * (jb + 1)]
        )

    # ---- qT per head -> block-diagonal qbd [128, NHP*2*32] ----
    # qbd cols: pair hp, batch b block of 32 = [side(2) x t(8), 16 zero-pad];
    # col = 64*hp + 32*b + 8*side + t ; rows = 64*side + d. Off-block zero.
    # The 16 zero-pad columns make each QK matmul M=32, so the four slot
    # matmuls (start=True each) cover all 128 partitions of the scores bank
    # and the full-bank mask matmul can accumulate (per-partition group rule).
    qbd = const.tile([128, NHP * BPC * 32], F32)
    nc.vector.memset(qbd[:], 0.0)
    qt_ps = psum.tile([128, 128], F32, tag="attnT", name="qt_ps")
    for h in range(NH):
        side = h % 2
        hp = h // 2
        nc.tensor.matmul(
            qt_ps[64 * side : 64 * side + 64, 16 * hp : 16 * hp + 16],
            lhsT=q_sb[:, HD * h : HD * (h + 1)],
            rhs=ident_sb[0 : BPC * T, 0 : BPC * T],
            start=True,
            stop=True,
        )
    for h in range(NH):
        side = h % 2
        hp = h // 2
        src = (
            qt_ps[64 * side : 64 * side + 64, :]
            .rearrange("p (hp b t) -> p hp b t", hp=NHP, b=BPC)[:, hp, :, :]
        )
        dst = (
            qbd[64 * side : 64 * side + 64, :]
            .rearrange("p (hp b s t) -> p hp b s t", hp=NHP, b=BPC, s=4)[:, hp, :, side, :]
        )
        nc.scalar.copy(dst, src)

    # ---- main loops over the two batches ----
    for b in range(BPC):
        kdram = k_in[b]
        vdram = v_in[b]

        attn_tiles = [[None] * 2 for _ in range(NCHUNK)]
        sums = [wk.tile([128, NCHUNK], F32, tag=f"sums{bk}", name=f"sums{b}_{bk}")
                for bk in range(2)]
        sc = [None, None]

        # -- pass 1: scores + exp per chunk --
        # k arrives pre-transposed from the host: kdram [NH, HD, S], so a kT
        # pair tile [128=(2h x 64d), 512 s] is one clean DMA (2KB contiguous
        # per partition) and no PE transposes or PSUM evacuations are needed.
        for c in range(NCHUNK):
            for hp in range(NHP):
                bk, slot = hp // 4, hp % 4
                kts = ktsb.tile([128, SCH], F32, tag="kts", name=f"kts{b}_{c}_{hp}")
                for side in range(2):
                    nc.sync.dma_start(
                        kts[64 * side : 64 * side + 64, :],
                        kdram[2 * hp + side][:, SCH * c : SCH * (c + 1)],
                    )

                if slot == 0:
                    sc[bk] = psum.tile([128, SCH], F32, tag="sc", bufs=4,
                                       name=f"sc{b}_{c}_{bk}")
                nc.tensor.matmul(
                    sc[bk][32 * slot : 32 * slot + 32, :],
                    lhsT=qbd[:, 64 * hp + 32 * b : 64 * hp + 32 * b + 32],
                    rhs=kts[:],
                    start=True,
                    stop=False,
                    tile_position=(0, 32 * slot),
                    # sim-only: the global group-check misaddresses
                    # partition-offset psum outs; data semantics are exact.
                    skip_group_check=True,
                )
                if slot == 3:
                    nc.tensor.matmul(
                        sc[bk][:],
                        lhsT=ones_sb[:],
                        rhs=mb_sb[0:1, S * b + SCH * c : S * b + SCH * (c + 1)],
                        start=False,
                        stop=True,
                        skip_group_check=True,
                    )
                    at = apool.tile([128, SCH], F32, tag="attn",
                                    name=f"attn{b}_{c}_{bk}")
                    nc.scalar.activation(
                        at[:], sc[bk][:], Exp, accum_out=sums[bk][:, c : c + 1]
                    )
                    attn_tiles[c][bk] = at

        # -- normalize --
        rcp = [wk.tile([128, 1], F32, tag=f"rcp{bk}", name=f"rcp{b}_{bk}")
               for bk in range(2)]
        for bk in range(2):
            tot = wk.tile([128, 1], F32, tag=f"tot{bk}", name=f"tot{b}_{bk}")
            nc.vector.reduce_sum(tot[:], sums[bk][:], axis=AxX)
            nc.vector.reciprocal(rcp[bk][:], tot[:])
        for c in range(NCHUNK):
            for bk in range(2):
                nc.vector.tensor_scalar_mul(
                    attn_tiles[c][bk][:], attn_tiles[c][bk][:], rcp[bk][:]
                )

        # -- pass 2: attn^T + AV --
        av = [
            psum.tile([T, 512], F32, tag="av", name=f"av{b}_{bk}")
            for bk in range(2)
        ]
        for c in range(NCHUNK):
            for h in range(NH):
                vt = vin.tile([128, NPIECE * HD], F32, tag="vin",
                              name=f"vt{b}_{c}_{h}")
                nc.sync.dma_start(
                    vt[:].rearrange("p (np d) -> p np d", np=NPIECE),
                    vdram[h].rearrange("(cc p) d -> p cc d", p=128)[
                        :, NPIECE * c : NPIECE * (c + 1), :
                    ],
                )
                if h == 0:
                    atps = [None, None]
                    att_sb = [None, None]
                    for bk in range(2):
                        atps[bk] = psum.tile([128, SCH], F32, tag="attnT",
                                             name=f"atp{b}_{c}_{bk}")
                        for p in range(NPIECE):
                            nc.tensor.matmul(
                                atps[bk][:, 128 * p : 128 * (p + 1)],
                                lhsT=attn_tiles[c][bk][:, 128 * p : 128 * (p + 1)],
                                rhs=ident_sb[:],
                                start=True,
                                stop=True,
                            )
                        att_sb[bk] = atp.tile([128, SCH], F32, tag="attnT_sb",
                                              name=f"atsb{b}_{c}_{bk}")
                        if bk == 0:
                            nc.scalar.copy(att_sb[bk][:], atps[bk][:])
                        else:
                            nc.vector.tensor_copy(att_sb[bk][:], atps[bk][:])
                hp, side = h // 2, h % 2
                bk, slot = hp // 4, hp % 4
                for p in range(NPIECE):
                    nc.tensor.matmul(
                        av[h // 8][:, 64 * (h % 8) : 64 * (h % 8) + 64],
                        lhsT=att_sb[bk][
                            :, 128 * p + 32 * slot + 8 * side : 128 * p + 32 * slot + 8 * side + 8
                        ],
                        rhs=vt[:, HD * p : HD * (p + 1)],
                        start=(c == 0 and p == 0 and h % 8 == 0),
                        stop=(c == NCHUNK - 1 and p == NPIECE - 1 and h % 8 == 7),
                    )

        # -- attn_out -> [(h d), t] -> out-proj --
        ao_sb = wk.tile([T, H], F32, tag="ao", name=f"ao{b}")
        for bk in range(2):
            nc.scalar.copy(ao_sb[:, 512 * bk : 512 * (bk + 1)], av[bk][:])
        aot_ps = psum.tile([128, HD], F32, tag="attnT", name=f"aotp{b}")
        for jc in range(8):
            nc.tensor.matmul(
                aot_ps[:, 8 * jc : 8 * (jc + 1)],
                lhsT=ao_sb[:, 128 * jc : 128 * (jc + 1)],
                rhs=ident_sb[0:T, 0:T],
                start=True,
                stop=True,
            )
        aot_sb = wk.tile([128, HD], F32, tag="aot", name=f"aot{b}")
        nc.scalar.copy(aot_sb[:], aot_ps[:])
        out_sb = wk.tile([T, H], F32, tag="out", name=f"out{b}")
        for ob in range(2):
            op = psum.tile([T, 512], F32, tag="av", name=f"outp{b}_{ob}")
            for jc in range(8):
                nc.tensor.matmul(
                    op[:],
                    lhsT=aot_sb[:, 8 * jc : 8 * (jc + 1)],
                    rhs=wot_sb[jc][:, 512 * ob : 512 * (ob + 1)],
                    start=(jc == 0),
                    stop=(jc == 7),
                )
            nc.vector.tensor_add(
                out_sb[:, 512 * ob : 512 * (ob + 1)], op[:],
                bo_sb[:, 512 * ob : 512 * (ob + 1)],
            )
        nc.sync.dma_start(outt[b], out_sb[:])


def _build():
    nc = bacc.Bacc("TRN2", target_bir_lowering=False, debug=False)
    ins = {}
    ins["xt"] = nc.dram_tensor("xt", [H, BPC * T], F32, kind="ExternalInput").ap()
    kin = []
    vin = []
    for b in range(BPC):
        kin.append(nc.dram_tensor(f"k{b}", [NH, HD, S], F32, kind="ExternalInput").ap())
        vin.append(nc.dram_tensor(f"v{b}", [NH, S, HD], F32, kind="ExternalInput").ap())
    ins["kin"] = kin
    ins["vin"] = vin
    ins["mb"] = nc.dram_tensor("mb", [1, BPC * S], F32, kind="ExternalInput").ap()
    ins["wqt"] = nc.dram_tensor("wqt", [H, H], F32, kind="ExternalInput").ap()
    ins["wot"] = nc.dram_tensor("wot", [H, H], F32, kind="ExternalInput").ap()
    ins["bqbc"] = nc.dram_tensor("bqbc", [BPC * T, H], F32, kind="ExternalInput").ap()
    ins["bobc"] = nc.dram_tensor("bobc", [T, H], F32, kind="ExternalInput").ap()
    ins["ident"] = nc.dram_tensor("ident", [128, 128], F32, kind="ExternalInput").ap()
    outt = nc.dram_tensor("outt", [BPC, T, H], F32, kind="ExternalOutput").ap()
    with tile.TileContext(nc) as tc:
        with ExitStack() as ctx:
            _emit(ctx, tc, ins, outt)
    nc.compile()
    return nc


_NC_CACHE = {}
LAST_RESULT = None


def _get_nc():
    if "nc" not in _NC_CACHE:
        _NC_CACHE["nc"] = _build()
    return _NC_CACHE["nc"]


def kernel(query_input, prev_key, prev_value, key_padding_mask, new_state_order,
           Wq, bq, Wo, bo):
    global LAST_RESULT
    nc = _get_nc()

    q = np.ascontiguousarray(np.asarray(query_input, dtype=np.float32))
    pk = np.asarray(prev_key)
    pv = np.asarray(prev_value)
    order = np.asarray(new_state_order).astype(np.int64)
    mask = np.asarray(key_padding_mask)

    wqt = np.ascontiguousarray(np.asarray(Wq, dtype=np.float32).T * np.float32(SCALE))
    wot = np.ascontiguousarray(np.asarray(Wo, dtype=np.float32).T)
    bqbc = np.ascontiguousarray(
        np.broadcast_to(np.asarray(bq, dtype=np.float32) * np.float32(SCALE),
                        (BPC * T, H))
    )
    bobc = np.ascontiguousarray(np.broadcast_to(np.asarray(bo, dtype=np.float32), (T, H)))
    ident = np.eye(128, dtype=np.float32)
    mbfull = (-np.float32(FMAX)) * mask.astype(np.float32)  # [16, 4096]

    in_maps = []
    for core in range(NCORES):
        bids = [BPC * core + i for i in range(BPC)]
        im = {
            "xt": np.ascontiguousarray(
                np.transpose(q[:, bids, :], (2, 1, 0)).reshape(H, BPC * T)
            ),
            "mb": np.ascontiguousarray(mbfull[bids].reshape(1, BPC * S)),
            "wqt": wqt, "wot": wot, "bqbc": bqbc, "bobc": bobc, "ident": ident,
        }
        for i, bi in enumerate(bids):
            # beam-reorder gather + [S, HD] -> [HD, S] transpose folded into
            # host-side shard staging; device DMAs kT tiles directly.
            im[f"k{i}"] = np.ascontiguousarray(
                pk[order[bi]].astype(np.float32, copy=False).transpose(0, 2, 1)
            )
            im[f"v{i}"] = np.ascontiguousarray(pv[order[bi]].astype(np.float32, copy=False))
        in_maps.append(im)

    res = run_bass_kernel_spmd(nc, in_maps, core_ids=list(range(NCORES)))
    LAST_RESULT = res

    out = np.empty((T, B, H), dtype=np.float32)
    for core in range(NCORES):
        o = res.results[core]["outt"]  # [BPC, T, H]
        for i in range(BPC):
            out[:, BPC * core + i, :] = o[i]
    return out
